# revision 7
# baseline (speedup 1.0000x reference)
"""Trainium2 Bass kernel for ContinuousConvEmbedding (Open3D-style continuous conv).

Math reformulation (validated vs reference to 1e-6 in fp32 / 5e-3 with bf16):
  For each output i / input j pair with rel = (p_in - p_out) * 2/extent:
    t_a = q_a + 1, q = rel * r/linf clipped;  the trilinear hat weights
    h_g(t) over the 3-bin grid are linear in (1, d, s) with d = t-1 = q,
    s = |d|:   h = M @ (1, d, s),  M = [[0,-.5,.5],[1,0,-1],[0,.5,.5]].
  So  sum_k A[i,j,k] kernel[k,c,f] = window * prod_axes(basis) contracted
  with a host-transformed kernel K2 = (M x M x M) kernel.  The window
  w = relu(1-r2)^3 is folded into the axis-0 basis.  Features are
  pre-contracted on the host: Phi[j,m,f] = sum_c features[j,c] K2[m,c,f].
  Device computes out[f,i] = sum_{j,m} m27[m;j,i] * Phi[j,m,f] via PE
  matmuls (contraction over j, accumulate over m in PSUM).

Sharding: output points i are sharded across the 8 cores (256 each);
the input cloud + Phi are replicated.  No collective needed.
"""
import sys

sys.path.insert(0, "/opt/trn_rl_repo")

import numpy as np
import ml_dtypes

import concourse.bass as bass
import concourse.mybir as mybir
import concourse.tile as tile
from concourse import bacc
from concourse.bass_utils import run_bass_kernel_spmd

F32 = mybir.dt.float32
BF16 = mybir.dt.bfloat16
AF = mybir.ActivationFunctionType
ALU = mybir.AluOpType

N_CORES = 8
N_IN = 2048
N_OUT = 2048
C_IN = 8
C_OUT = 64
K3 = 27
EPS = 1e-12

NI = N_OUT // N_CORES          # 256 output points per core
NJT = N_IN // 128              # 16 j-tiles
JT_PER_Q = 2                   # j-tiles per geometry block
NQ = NJT // JT_PER_Q           # 4 geometry blocks
FQ = JT_PER_Q * NI             # 1024 free-dim elements per geometry op

# h_g(t) = sum_m M_BASIS[g, m] * (1, d, s)[m]
M_BASIS = np.array([[0.0, -0.5, 0.5],
                    [1.0, 0.0, -1.0],
                    [0.0, 0.5, 0.5]], np.float32)


CHOP = 256  # DVE ops are emitted in free-dim chunks of this size (None = whole op)


def _chunks(total, step):
    if step is None:
        return [slice(0, total)]
    return [slice(o, min(o + step, total)) for o in range(0, total, step)]


def build_nc(repeat: int = 1, chop=None):
    """Build the SPMD bass program (same on every core)."""
    if chop is None:
        chop = CHOP
    nc = bacc.Bacc("TRN2", target_bir_lowering=False, debug=False,
                   num_devices=N_CORES)
    phi_d = nc.dram_tensor("phi", [N_IN, K3 * C_OUT], BF16, kind="ExternalInput").ap()
    lhsT5_d = nc.dram_tensor("lhsT5", [5, N_IN], F32, kind="ExternalInput").ap()
    rhs5_d = nc.dram_tensor("rhs5", [5, NI], F32, kind="ExternalInput").ap()
    nbcast_d = nc.dram_tensor("nbcast", [128, 3 * NI], F32, kind="ExternalInput").ap()
    pin_d = nc.dram_tensor("pin_sc", [128, NJT * 3], F32, kind="ExternalInput").ap()
    bias_d = nc.dram_tensor("bias", [C_OUT, 1], F32, kind="ExternalInput").ap()
    y_d = nc.dram_tensor("y", [C_OUT, NI], F32, kind="ExternalOutput").ap()

    with tile.TileContext(nc) as tc:
        with tc.tile_pool(name="const", bufs=1) as constp, \
             tc.tile_pool(name="phip", bufs=1) as phip, \
             tc.tile_pool(name="geo", bufs=2) as geo, \
             tc.tile_pool(name="bfp", bufs=2) as bfp, \
             tc.tile_pool(name="rhsp", bufs=6) as rhsp, \
             tc.tile_pool(name="outp", bufs=2) as outp, \
             tc.tile_pool(name="psr2", bufs=2, space="PSUM") as psr2, \
             tc.tile_pool(name="psout", bufs=1, space="PSUM") as psout:

            # ---- constants / inputs resident in SBUF ----
            lhsT5 = constp.tile([5, N_IN], F32)
            rhs5 = constp.tile([5, NI], F32)
            nbcast = constp.tile([128, 3 * NI], F32)
            pin = constp.tile([128, NJT * 3], F32)
            bias = constp.tile([C_OUT, 1], F32)
            nc.sync.dma_start(lhsT5[:], lhsT5_d[:])
            nc.sync.dma_start(rhs5[:], rhs5_d[:])
            nc.sync.dma_start(nbcast[:], nbcast_d[:])
            nc.sync.dma_start(pin[:], pin_d[:])
            nc.sync.dma_start(bias[:], bias_d[:])

            # Phi resident: [128 part, jt, m*64], j = jt*128 + p
            phi = phip.tile([128, NJT, K3 * C_OUT], BF16)
            phi_ap = phi_d.rearrange("(a p) x -> p a x", p=128)
            for q in range(NQ):
                jts = slice(q * JT_PER_Q, (q + 1) * JT_PER_Q)
                nc.sync.dma_start(phi[:, jts, :], phi_ap[:, jts, :])

            out_acc = psout.tile([C_OUT, NI], F32)

            def vtt(out, a, b, op):
                step = chop if chop is None else (chop * 2 if out.dtype == BF16 else chop)
                for sl in _chunks(out.shape[-1], step):
                    nc.vector.tensor_tensor(out[:, sl], a[:, sl], b[:, sl], op)

            def vrecip(out, a):
                for sl in _chunks(out.shape[-1], chop):
                    nc.vector.reciprocal_approx_fast(out[:, sl], a[:, sl])

            def body(_iv=None):
                n_mm = 0
                tot_mm = NQ * K3 * JT_PER_Q
                for q in range(NQ):
                    # ---- geometry for j-tiles q*4 .. q*4+3, free dim (jl, i) ----
                    r2q = psr2.tile([128, FQ], F32, tag="r2q", name="r2q")
                    rel = [geo.tile([128, FQ], F32, tag=f"rel{a}", name=f"rel{a}") for a in range(3)]
                    for jl in range(JT_PER_Q):
                        jt = q * JT_PER_Q + jl
                        cs = slice(jl * NI, (jl + 1) * NI)
                        nc.tensor.matmul(r2q[:, cs],
                                         lhsT5[:, jt * 128:(jt + 1) * 128],
                                         rhs5[:], start=True, stop=True)
                        for a in range(3):
                            nc.vector.tensor_scalar_add(
                                rel[a][:, cs],
                                nbcast[:, a * NI:(a + 1) * NI],
                                pin[:, jt * 3 + a: jt * 3 + a + 1])
                    # window base u = relu(1 - r2); r = sqrt(r2 + eps)
                    u = geo.tile([128, FQ], F32, tag="u", name="u")
                    nc.scalar.activation(u[:], r2q[:], AF.Relu, bias=1.0, scale=-1.0)
                    r2p = geo.tile([128, FQ], F32, tag="r2p", name="r2p")
                    nc.scalar.activation(r2p[:], r2q[:], AF.Relu)
                    r_ = geo.tile([128, FQ], F32, tag="r", name="r_")
                    nc.scalar.activation(r_[:], r2p[:], AF.Sqrt)
                    # linf = max_a |rel_a|; rl = 1/(linf+eps); scale = r*rl
                    ab = []
                    for a in range(3):
                        aba = geo.tile([128, FQ], F32, tag=f"ab{a}", name=f"ab{a}")
                        nc.scalar.activation(aba[:], rel[a][:], AF.Abs)
                        ab.append(aba)
                    linf1 = geo.tile([128, FQ], F32, tag="linf1", name="linf1")
                    vtt(linf1, ab[0], ab[1], ALU.max)
                    linf = geo.tile([128, FQ], F32, tag="linf", name="linf")
                    vtt(linf, linf1, ab[2], ALU.max)
                    for _sl in _chunks(FQ, chop):
                        nc.vector.tensor_scalar_add(linf[:, _sl], linf[:, _sl], EPS)
                    rl = geo.tile([128, FQ], F32, tag="rl", name="rl")
                    vrecip(rl, linf)
                    scale = geo.tile([128, FQ], F32, tag="scale", name="scale")
                    vtt(scale, r_, rl, ALU.mult)
                    # d_a = rel_a * scale (bf16), s_a = |d_a|
                    d = []
                    s = []
                    for a in range(3):
                        da = bfp.tile([128, FQ], BF16, tag=f"d{a}", name=f"d{a}")
                        vtt(da, rel[a], scale, ALU.mult)
                        d.append(da)
                        sa = bfp.tile([128, FQ], BF16, tag=f"s{a}", name=f"s{a}")
                        nc.scalar.activation(sa[:], da[:], AF.Abs)
                        s.append(sa)
                    # window w = u^3 (bf16)
                    u2 = geo.tile([128, FQ], F32, tag="u2", name="u2")
                    nc.scalar.activation(u2[:], u[:], AF.Square)
                    w = bfp.tile([128, FQ], BF16, tag="w", name="w")
                    vtt(w, u2, u, ALU.mult)
                    # axis-0 basis with window folded in
                    wd0 = bfp.tile([128, FQ], BF16, tag="wd0", name="wd0")
                    vtt(wd0, w, d[0], ALU.mult)
                    ws0 = bfp.tile([128, FQ], BF16, tag="ws0", name="ws0")
                    vtt(ws0, w, s[0], ALU.mult)
                    b0 = [w, wd0, ws0]
                    # expansion tree + matmuls
                    for m0 in range(3):
                        for m1 in range(3):
                            if m1 == 0:
                                zy = b0[m0]
                            else:
                                g1 = d[1] if m1 == 1 else s[1]
                                zy = bfp.tile([128, FQ], BF16, tag="zy", name="zy")
                                vtt(zy, b0[m0], g1, ALU.mult)
                            for m2 in range(3):
                                if m2 == 0:
                                    rhs_t = zy
                                else:
                                    g2 = d[2] if m2 == 1 else s[2]
                                    rhs_t = rhsp.tile([128, FQ], BF16, tag="rhs", name="rhs_t")
                                    vtt(rhs_t, zy, g2, ALU.mult)
                                m = m0 * 9 + m1 * 3 + m2
                                for jl in range(JT_PER_Q):
                                    jt = q * JT_PER_Q + jl
                                    nc.tensor.matmul(
                                        out_acc[:],
                                        phi[:, jt, m * C_OUT:(m + 1) * C_OUT],
                                        rhs_t[:, jl * NI:(jl + 1) * NI],
                                        start=(n_mm == 0), stop=(n_mm == tot_mm - 1))
                                    n_mm += 1
                # out = relu(acc + bias), DMA out
                out_sb = outp.tile([C_OUT, NI], F32, tag="out", name="out_sb")
                nc.scalar.activation(out_sb[:], out_acc[:], AF.Relu, bias=bias[:, 0:1])
                nc.sync.dma_start(y_d[:], out_sb[:])

            if repeat == 1:
                body()
            else:
                with tc.For_i(0, repeat, 1,
                              hint_engines=(mybir.EngineType.PE,)) as iv:
                    body(iv)
    nc.compile()
    return nc


def host_prep(features, pos_input, pos_output, extents, kernel, bias):
    """Host-side preprocessing -> per-core input maps."""
    features = np.asarray(features, np.float32)
    pos_input = np.asarray(pos_input, np.float32)
    pos_output = np.asarray(pos_output, np.float32)
    kernel = np.asarray(kernel, np.float32)
    bias = np.asarray(bias, np.float32)
    sc = 2.0 / float(np.asarray(extents).reshape(-1)[0])
    pin = pos_input * sc
    pout = pos_output * sc

    K5 = kernel.reshape(3, 3, 3, C_IN, C_OUT)
    K2 = np.einsum("am,bn,co,abcuf->mnouf", M_BASIS, M_BASIS, M_BASIS, K5)
    # Phi[j, m, f] = features @ K2
    phi = features @ K2.reshape(27, C_IN, C_OUT).transpose(1, 0, 2).reshape(C_IN, -1)
    phi = phi.reshape(N_IN, K3 * C_OUT).astype(ml_dtypes.bfloat16)

    pin_n2 = np.sum(pin * pin, -1)
    lhsT5 = np.stack([pin_n2, pin[:, 0], pin[:, 1], pin[:, 2],
                      np.ones(N_IN, np.float32)]).astype(np.float32)
    pin_sc = pin.reshape(NJT, 128, 3).transpose(1, 0, 2).reshape(128, NJT * 3)
    pin_sc = np.ascontiguousarray(pin_sc, np.float32)
    bias_col = bias.reshape(C_OUT, 1).astype(np.float32)

    in_maps = []
    for c in range(N_CORES):
        po = pout[c * NI:(c + 1) * NI]
        po_n2 = np.sum(po * po, -1)
        rhs5 = np.stack([np.ones(NI, np.float32), -2.0 * po[:, 0],
                         -2.0 * po[:, 1], -2.0 * po[:, 2], po_n2]).astype(np.float32)
        nbcast = np.tile(np.concatenate([-po[:, 0], -po[:, 1], -po[:, 2]])[None, :],
                         (128, 1)).astype(np.float32)
        in_maps.append({
            "phi": phi, "lhsT5": lhsT5, "rhs5": rhs5,
            "nbcast": nbcast, "pin_sc": pin_sc, "bias": bias_col,
        })
    return in_maps


_NC_CACHE = {}


def _get_nc(repeat=1, chop=None):
    key = (repeat, chop if chop is not None else CHOP)
    if key not in _NC_CACHE:
        _NC_CACHE[key] = build_nc(repeat, chop)
    return _NC_CACHE[key]


def kernel(features, pos_input, pos_output, extents, kernel, bias):
    nc = _get_nc(1)
    in_maps = host_prep(features, pos_input, pos_output, extents, kernel, bias)
    res = run_bass_kernel_spmd(nc, in_maps, core_ids=list(range(N_CORES)),
                               trace=False)
    out = np.concatenate([res.results[c]["y"] for c in range(N_CORES)], axis=1)
    return np.ascontiguousarray(out.T, dtype=np.float32)
